# revision 5
# baseline (speedup 1.0000x reference)
# Trainium2 Bass kernel for EndPointRepr (span endpoint representations).
#
# reference:
#   h = encoded_input @ W + b                    # [B, S, P]
#   res_k[q] = concat(h[qb[q], s_k[q]], h[qb[q], e_k[q]]) * (e_k[q] >= s_k[q])
#
# Sharding: data-parallel over batch. Core c owns batch c; the host routes
# each valid (e >= s) query to its batch's core. Invalid queries are never
# routed; the host-side result buffers start zeroed.
#
# Device pipeline (bf16 data path, fp32 PSUM accumulation), all layouts
# pre-packed on the host for contiguous big-descriptor DMAs:
#   phase 1: hT = W.T @ X.T chunk by chunk: W k-blocks stationary (only 64
#            LDWEIGHTS), X.T chunks moving, accumulating hT [p', s] in PSUM.
#            DVE/ACT alternate folding the per-partition bias while packing
#            PSUM into hI [128, S, 2] bf16 (feature 128*i + p on (p, ..., i)).
#            A few identity warmup matmuls during the NEFF preamble ramp the
#            PE out of its low-clock p-state before the real work lands.
#   phase 2: one GpSimd ap_gather per stream pulls the s- and e-endpoint
#            columns of hI (free-axis gather, 8 Q7 cores in parallel, no DMA
#            descriptors), then one straight contiguous DMA per stream writes
#            the [128, 2*QCAP, 2] result to DRAM. The host un-transposes.
# bf16 keeps the PE at 1 cycle/row (fp32 is 4) and halves all DMA traffic;
# rel err ~3e-3 against the fp32 reference, well inside the 2e-2 gate.
import numpy as np

B, S, D, P = 8, 2048, 1024, 256
NQ = 8192
NCORES = 8
KB = D // 128          # contraction k-blocks
PB = P // 128          # feature partition-blocks
QCAP = 640             # per-endpoint gather capacity (multiple of 16)
SCH = 4                # xT chunks along S for DMA/matmul overlap
SCHW = S // SCH
NWARM = 32             # PE warmup matmuls (p-state ramp during preamble)

_cache = {}


def _build_nc():
    import concourse.bacc as bacc
    import concourse.mybir as mybir
    import concourse.tile as tile
    from concourse.masks import make_identity

    f32 = mybir.dt.float32
    bf16 = mybir.dt.bfloat16
    nc = bacc.Bacc("TRN2", target_bir_lowering=False, debug=False,
                   num_devices=NCORES)

    xh = nc.dram_tensor("xh", [128, SCH * KB * SCHW], bf16,
                        kind="ExternalInput").ap()
    wh = nc.dram_tensor("wh", [128, KB * P], bf16, kind="ExternalInput").ap()
    biasc = nc.dram_tensor("biasc", [128, PB], f32,
                           kind="ExternalInput").ap()
    idx = nc.dram_tensor("idx", [128, 4 * QCAP // 16], mybir.dt.int16,
                         kind="ExternalInput").ap()
    rt1 = nc.dram_tensor("rt1", [128, 2 * QCAP, 2], bf16,
                         kind="ExternalOutput").ap()
    rt2 = nc.dram_tensor("rt2", [128, 2 * QCAP, 2], bf16,
                         kind="ExternalOutput").ap()

    with tile.TileContext(nc) as tc:
        with (
            tc.tile_pool(name="consts", bufs=1) as consts,
            tc.tile_pool(name="xin", bufs=2) as xt_pool,
            tc.tile_pool(name="gath", bufs=1) as g_pool,
            tc.tile_pool(name="ps", bufs=4, space="PSUM") as ps_pool,
            tc.tile_pool(name="warm", bufs=1, space="PSUM") as warm_pool,
        ):
            identity = consts.tile([128, 128], bf16)
            make_identity(nc, identity)
            warm_ps = warm_pool.tile([128, 128], f32)
            for _ in range(NWARM):
                nc.tensor.matmul(warm_ps, identity, identity,
                                 start=True, stop=True)

            w_sb = consts.tile([128, KB, P], bf16)
            nc.scalar.dma_start(w_sb,
                                wh.rearrange("p (kb j) -> p kb j", kb=KB))
            biasc_sb = consts.tile([128, PB], f32)
            nc.scalar.dma_start(biasc_sb, biasc)
            idx_sb = consts.tile([128, 4 * QCAP // 16], mybir.dt.int16)
            nc.scalar.dma_start(idx_sb, idx)

            hI = consts.tile([128, S, PB], bf16, name="hI")

            xh_view = xh.rearrange("p (c kb s) -> p c kb s", c=SCH, kb=KB)
            for c in range(SCH):
                xt_c = xt_pool.tile([128, KB, SCHW], bf16, tag="xt")
                nc.sync.dma_start(xt_c, xh_view[:, c])
                for pblk in range(PB):
                    hT_ps = ps_pool.tile([128, SCHW], f32, tag="hps")
                    for kb in range(KB):
                        nc.tensor.matmul(
                            hT_ps, w_sb[:, kb, pblk * 128:(pblk + 1) * 128],
                            xt_c[:, kb, :],
                            start=(kb == 0), stop=(kb == KB - 1))
                    out_ap = hI[:, c * SCHW:(c + 1) * SCHW, pblk]
                    bias_ap = biasc_sb[:, pblk:pblk + 1]
                    if (c * PB + pblk) % 2 == 0:
                        nc.vector.tensor_scalar_add(out_ap, hT_ps, bias_ap)
                    else:
                        nc.scalar.activation(
                            out_ap, hT_ps,
                            mybir.ActivationFunctionType.Identity,
                            bias=bias_ap, scale=1.0)

            for st, r in ((0, rt1), (1, rt2)):
                g_sb = g_pool.tile([128, 2 * QCAP, PB], bf16, name=f"g{st}")
                nc.gpsimd.ap_gather(
                    g_sb, hI[:, :, :],
                    idx_sb[:, st * (2 * QCAP // 16):
                           (st + 1) * (2 * QCAP // 16)],
                    channels=128, num_elems=S, d=PB, num_idxs=2 * QCAP)
                nc.scalar.dma_start(r, g_sb)

    nc.compile()
    return nc


def _get_nc():
    if "nc" not in _cache:
        _cache["nc"] = _build_nc()
    return _cache["nc"]


def _numpy_ref(flag, encoded_input, start_ids_1, end_ids_1, query_batch_idx,
               start_ids_2, end_ids_2, W, b):
    h = encoded_input.astype(np.float32) @ W.astype(np.float32) + \
        b.astype(np.float32)
    qb = np.asarray(query_batch_idx).astype(np.int64)

    def span(s, e):
        s = np.asarray(s).astype(np.int64)
        e = np.asarray(e).astype(np.int64)
        rep = np.concatenate([h[qb, s], h[qb, e]], axis=-1)
        return rep * (e >= s)[:, None].astype(rep.dtype)

    return span(start_ids_1, end_ids_1), span(start_ids_2, end_ids_2)


def kernel(flag, encoded_input, start_ids_1, end_ids_1, query_batch_idx,
           start_ids_2, end_ids_2, W, b):
    import ml_dtypes
    from concourse.bass_utils import run_bass_kernel_spmd

    bf16 = ml_dtypes.bfloat16
    x_full = np.asarray(encoded_input, dtype=np.float32)
    w_np = np.asarray(W, dtype=np.float32)
    b_np = np.asarray(b).astype(np.float32)
    qb = np.asarray(query_batch_idx).astype(np.int64)
    s1 = np.asarray(start_ids_1).astype(np.int64)
    e1 = np.asarray(end_ids_1).astype(np.int64)
    s2 = np.asarray(start_ids_2).astype(np.int64)
    e2 = np.asarray(end_ids_2).astype(np.int64)

    in_range = (qb.min() >= 0 and qb.max() < B and
                all(a.min() >= 0 and a.max() < S for a in (s1, e1, s2, e2)))

    in_maps, ids_all = [], []
    try:
        if not in_range or x_full.shape != (B, S, D):
            raise ValueError("shape/range")
        # wh[p, kb*P + j] = W[kb*128 + p, j]
        wh = np.ascontiguousarray(
            w_np.reshape(KB, 128, P).transpose(1, 0, 2).reshape(128, KB * P)
        ).astype(bf16)
        # biasc[p, i] = b[i*128 + p]
        biasc = np.ascontiguousarray(b_np.reshape(PB, 128).T,
                                     dtype=np.float32)
        for bb in range(B):
            sel = qb == bb
            idx_w = np.zeros((2, 2 * QCAP), np.int16)
            ids_pair = []
            for pi, (s, e) in enumerate([(s1, e1), (s2, e2)]):
                ids = np.nonzero(sel & (e >= s))[0]
                if len(ids) > QCAP:
                    raise ValueError("capacity overflow")
                ids_pair.append(ids)
                n = len(ids)
                idx_w[pi, :n] = s[ids]
                idx_w[pi, QCAP:QCAP + n] = e[ids]
            ids_all.append(ids_pair)
            # wrap each stream's slots: slot j -> (partition j%16, col j//16)
            idx_wr = np.concatenate(
                [idx_w[pi].reshape(2 * QCAP // 16, 16).T for pi in range(2)],
                axis=1)
            idx_wr = np.ascontiguousarray(np.tile(idx_wr, (8, 1)))
            # xh[p, ((c*KB)+kb)*SCHW + s'] = X[c*SCHW + s', kb*128 + p]
            xr = x_full[bb].reshape(SCH, SCHW, KB, 128) \
                .transpose(3, 0, 2, 1).reshape(128, SCH * KB * SCHW)
            in_maps.append({
                "xh": np.ascontiguousarray(xr).astype(bf16),
                "wh": wh,
                "biasc": biasc,
                "idx": idx_wr,
            })
    except ValueError:
        res1, res2 = _numpy_ref(flag, x_full, s1, e1, qb, s2, e2, w_np, b_np)
        return np.asarray(res1, np.float32), np.asarray(res2, np.float32)

    nc = _get_nc()
    out = run_bass_kernel_spmd(nc, in_maps, core_ids=list(range(NCORES)))
    _cache["last_run"] = out

    res1 = np.zeros((NQ, 2 * P), np.float32)
    res2 = np.zeros((NQ, 2 * P), np.float32)
    for bb in range(B):
        for pi, (res, key) in enumerate([(res1, "rt1"), (res2, "rt2")]):
            ids = ids_all[bb][pi]
            n = len(ids)
            if n == 0:
                continue
            rt = np.asarray(out.results[bb][key]).astype(np.float32)
            # rt[p, j, i] = h[idx_j, i*128 + p] -> feature-major [2*P, slots]
            flat = rt.transpose(2, 0, 1).reshape(P, 2 * QCAP)
            res[ids] = np.concatenate(
                [flat[:, :n].T, flat[:, QCAP:QCAP + n].T], axis=1)
    return res1, res2


# revision 6
# speedup vs baseline: 1.7506x; 1.7506x over previous
# Trainium2 Bass kernel for EndPointRepr (span endpoint representations).
#
# reference:
#   h = encoded_input @ W + b                    # [B, S, P]
#   res_k[q] = concat(h[qb[q], s_k[q]], h[qb[q], e_k[q]]) * (e_k[q] >= s_k[q])
#
# Sharding: data-parallel over batch. Core c owns batch c; the host routes
# each valid (e >= s) query to its batch's core. Invalid queries are never
# routed; the host-side result buffers start zeroed.
#
# Device pipeline (bf16 data path, fp32 PSUM accumulation):
#   The host compacts the batch to only the h rows some query references
#   (~1350 of 2048, capacity HROWS), remaps the indices, and sorts each
#   stream's queries by their max referenced row so gathers can chase the
#   matmul. All inputs are host-packed for contiguous large-descriptor DMAs.
#   phase 1: per 128-row block, 8 k-block matmuls (xT tile stationary, W
#            moving) accumulate h in PSUM; DVE folds the bias in while
#            down-casting to a bf16 SBUF tile, which spills to a DRAM h
#            scratch. Identity warmup matmuls during the NEFF preamble ramp
#            the PE out of its low-clock p-state.
#   phase 2: per (stream, endpoint) and per 128-query tile, one hardware-DGE
#            indirect DMA (dynamic per-partition row offsets, no GpSimd
#            ucode) gathers endpoint rows DRAM->SBUF; a second DMA writes the
#            [128, P] tile into its slice of the natural [QCAP, 2P] result.
#            Sorted queries + explicit deps onto the h spills let each tile
#            fire as soon as its h prefix has landed.
# bf16 keeps the PE at 1 cycle/row (fp32 is 4) and halves all DMA traffic;
# rel err ~3e-3 against the fp32 reference, well inside the 2e-2 gate.
import numpy as np

B, S, D, P = 8, 2048, 1024, 256
NQ = 8192
NCORES = 8
KB = D // 128          # contraction k-blocks
HROWS = 1536           # compacted h row capacity (multiple of SCHW)
HB = HROWS // 128      # h row blocks
QCAP = 640             # per-endpoint query capacity (multiple of 128)
QT = QCAP // 128       # query tiles per stream-endpoint
SCH = 3                # x chunks for DMA/matmul overlap
SCHW = HROWS // SCH
NWARM = 32             # PE warmup matmuls (p-state ramp during preamble)

_cache = {}


def _build_nc():
    import concourse.bacc as bacc
    import concourse.mybir as mybir
    import concourse.tile as tile
    from concourse.masks import make_identity
    from concourse.tile import add_dep_helper
    from concourse.bass import IndirectOffsetOnAxis

    f32 = mybir.dt.float32
    bf16 = mybir.dt.bfloat16
    nc = bacc.Bacc("TRN2", target_bir_lowering=False, debug=False,
                   num_devices=NCORES)

    xh = nc.dram_tensor("xh", [128, SCH * KB * SCHW], bf16,
                        kind="ExternalInput").ap()
    wh = nc.dram_tensor("wh", [128, KB * P], bf16, kind="ExternalInput").ap()
    bias = nc.dram_tensor("bias", [128, P], f32, kind="ExternalInput").ap()
    off = nc.dram_tensor("off", [128, 4 * QT], mybir.dt.int32,
                         kind="ExternalInput").ap()
    r1 = nc.dram_tensor("r1", [QCAP, 2 * P], bf16, kind="ExternalOutput").ap()
    r2 = nc.dram_tensor("r2", [QCAP, 2 * P], bf16, kind="ExternalOutput").ap()
    h_dram = nc.dram_tensor("h_scratch", [HROWS, P], bf16).ap()

    with tile.TileContext(nc) as tc:
        with (
            tc.tile_pool(name="consts", bufs=1) as consts,
            tc.tile_pool(name="xin", bufs=2) as xt_pool,
            tc.tile_pool(name="hsb", bufs=3) as h_pool,
            tc.tile_pool(name="gath", bufs=1) as g_pool,
            tc.tile_pool(name="ps", bufs=4, space="PSUM") as ps_pool,
            tc.tile_pool(name="warm", bufs=1, space="PSUM") as warm_pool,
        ):
            identity = consts.tile([128, 128], bf16)
            make_identity(nc, identity)
            warm_ps = warm_pool.tile([128, 128], f32)
            for _ in range(NWARM):
                nc.tensor.matmul(warm_ps, identity, identity,
                                 start=True, stop=True)

            w_sb = consts.tile([128, KB, P], bf16)
            nc.scalar.dma_start(w_sb,
                                wh.rearrange("p (kb j) -> p kb j", kb=KB))
            bias_sb = consts.tile([128, P], f32)
            nc.scalar.dma_start(bias_sb, bias)
            off_sb = consts.tile([128, 4 * QT], mybir.dt.int32)
            nc.scalar.dma_start(off_sb, off)

            # phase 1: h = X @ W + b, one [128, P] row-block at a time
            h_writes = []
            xh_view = xh.rearrange("p (c kb s) -> p c kb s", c=SCH, kb=KB)
            for c in range(SCH):
                xt_c = xt_pool.tile([128, KB, SCHW], bf16, tag="xt")
                nc.sync.dma_start(xt_c, xh_view[:, c])
                for ml in range(SCHW // 128):
                    m = c * (SCHW // 128) + ml
                    h_ps = ps_pool.tile([128, P], f32, tag="hps")
                    for kb in range(KB):
                        nc.tensor.matmul(
                            h_ps, xt_c[:, kb, ml * 128:(ml + 1) * 128],
                            w_sb[:, kb, :],
                            start=(kb == 0), stop=(kb == KB - 1))
                    h_sb = h_pool.tile([128, P], bf16, tag="h")
                    nc.vector.tensor_add(h_sb, h_ps, bias_sb)
                    h_writes.append(
                        nc.sync.dma_start(
                            h_dram[m * 128:(m + 1) * 128, :], h_sb))

            # phase 2: HW-DGE indirect gathers, one [128, P] tile at a time.
            # mb_need[st*QT + t] (host-computed, baked in via closure) isn't
            # available at build time, so depend on all spills per tile via
            # the conservative schedule: tile t of any stream waits for the
            # first (t + 1) * HB // QT blocks... instead we wire deps from a
            # static map passed through _cache (set before _build_nc()).
            mb_need = _cache["mb_need"]  # [4 * QT] ints: spills needed
            for st, (r, col0) in enumerate(
                    [(r1, 0), (r1, P), (r2, 0), (r2, P)]):
                g_sb = g_pool.tile([128, QT, P], bf16, name=f"g{st}")
                out_view = r.rearrange("(t p) c -> p t c", p=128)
                for t in range(QT):
                    j = st * QT + t
                    gi = nc.gpsimd.indirect_dma_start(
                        out=g_sb[:, t, :],
                        out_offset=None,
                        in_=h_dram[:, :],
                        in_offset=IndirectOffsetOnAxis(
                            ap=off_sb[:, j:j + 1], axis=0),
                    )
                    for m in range(mb_need[j]):
                        add_dep_helper(gi.ins, h_writes[m].ins,
                                       reason=f"gather st{st} t{t} reads h")
                    nc.scalar.dma_start(
                        out_view[:, t:t + 1, col0:col0 + P],
                        g_sb[:, t:t + 1, :])

    nc.compile()
    return nc


def _get_nc(mb_need):
    key = ("nc", tuple(mb_need))
    if key not in _cache:
        _cache["mb_need"] = mb_need
        _cache[key] = _build_nc()
    return _cache[key]


def _numpy_ref(flag, encoded_input, start_ids_1, end_ids_1, query_batch_idx,
               start_ids_2, end_ids_2, W, b):
    h = encoded_input.astype(np.float32) @ W.astype(np.float32) + \
        b.astype(np.float32)
    qb = np.asarray(query_batch_idx).astype(np.int64)

    def span(s, e):
        s = np.asarray(s).astype(np.int64)
        e = np.asarray(e).astype(np.int64)
        rep = np.concatenate([h[qb, s], h[qb, e]], axis=-1)
        return rep * (e >= s)[:, None].astype(rep.dtype)

    return span(start_ids_1, end_ids_1), span(start_ids_2, end_ids_2)


def kernel(flag, encoded_input, start_ids_1, end_ids_1, query_batch_idx,
           start_ids_2, end_ids_2, W, b):
    import ml_dtypes
    from concourse.bass_utils import run_bass_kernel_spmd

    bf16 = ml_dtypes.bfloat16
    x_full = np.asarray(encoded_input, dtype=np.float32)
    w_np = np.asarray(W, dtype=np.float32)
    b_np = np.asarray(b).astype(np.float32)
    qb = np.asarray(query_batch_idx).astype(np.int64)
    s1 = np.asarray(start_ids_1).astype(np.int64)
    e1 = np.asarray(end_ids_1).astype(np.int64)
    s2 = np.asarray(start_ids_2).astype(np.int64)
    e2 = np.asarray(end_ids_2).astype(np.int64)

    in_range = (qb.min() >= 0 and qb.max() < B and
                all(a.min() >= 0 and a.max() < S for a in (s1, e1, s2, e2)))

    in_maps, ids_all = [], []
    # per-(core, stream-endpoint, tile) h-block prefix needed; max over cores
    mb_need = np.ones(4 * QT, np.int64)
    try:
        if not in_range or x_full.shape != (B, S, D):
            raise ValueError("shape/range")
        wh = np.ascontiguousarray(
            w_np.reshape(KB, 128, P).transpose(1, 0, 2).reshape(128, KB * P)
        ).astype(bf16)
        bias_rep = np.ascontiguousarray(
            np.broadcast_to(b_np[None, :], (128, P)), dtype=np.float32)
        for bb in range(B):
            sel = qb == bb
            ids1 = np.nonzero(sel & (e1 >= s1))[0]
            ids2 = np.nonzero(sel & (e2 >= s2))[0]
            if len(ids1) > QCAP or len(ids2) > QCAP:
                raise ValueError("capacity overflow")
            rows = np.unique(np.concatenate(
                [s1[ids1], e1[ids1], s2[ids2], e2[ids2],
                 np.zeros(1, np.int64)]))
            if len(rows) > HROWS:
                raise ValueError("row overflow")
            # compact row ids, ascending original order
            cs1 = np.searchsorted(rows, s1[ids1])
            ce1 = np.searchsorted(rows, e1[ids1])
            cs2 = np.searchsorted(rows, s2[ids2])
            ce2 = np.searchsorted(rows, e2[ids2])
            # sort queries by max referenced compact row
            o1 = np.argsort(np.maximum(cs1, ce1), kind="stable")
            o2 = np.argsort(np.maximum(cs2, ce2), kind="stable")
            ids1, cs1, ce1 = ids1[o1], cs1[o1], ce1[o1]
            ids2, cs2, ce2 = ids2[o2], cs2[o2], ce2[o2]
            ids_all.append((ids1, ids2))
            off_np = np.zeros((4, QCAP), np.int32)
            for j, carr in enumerate([cs1, ce1, cs2, ce2]):
                off_np[j, :len(carr)] = carr
            # spill blocks needed per tile (pad slots point at row 0)
            for st in range(4):
                for t in range(QT):
                    seg = off_np[st, t * 128:(t + 1) * 128]
                    mb = int(seg.max()) // 128 + 1
                    mb_need[st * QT + t] = max(mb_need[st * QT + t], mb)
            off_w = np.ascontiguousarray(
                off_np.reshape(4, QT, 128).transpose(2, 0, 1)
                .reshape(128, 4 * QT))
            # compacted, padded X rows -> xh[p, c, kb, s']
            xc = np.zeros((HROWS, D), np.float32)
            xc[:len(rows)] = x_full[bb][rows]
            xr = xc.reshape(SCH, SCHW, KB, 128).transpose(3, 0, 2, 1) \
                .reshape(128, SCH * KB * SCHW)
            in_maps.append({
                "xh": np.ascontiguousarray(xr).astype(bf16),
                "wh": wh,
                "bias": bias_rep,
                "off": off_w,
            })
    except ValueError:
        res1, res2 = _numpy_ref(flag, x_full, s1, e1, qb, s2, e2, w_np, b_np)
        return np.asarray(res1, np.float32), np.asarray(res2, np.float32)

    nc = _get_nc(tuple(int(v) for v in mb_need))
    out = run_bass_kernel_spmd(nc, in_maps, core_ids=list(range(NCORES)))
    _cache["last_run"] = out

    res1 = np.zeros((NQ, 2 * P), np.float32)
    res2 = np.zeros((NQ, 2 * P), np.float32)
    for bb in range(B):
        for ids, res, key in [(ids_all[bb][0], res1, "r1"),
                              (ids_all[bb][1], res2, "r2")]:
            n = len(ids)
            if n:
                r = np.asarray(out.results[bb][key]).astype(np.float32)
                res[ids] = r[:n]
    return res1, res2
